# revision 14
# baseline (speedup 1.0000x reference)
"""Trainium2 Bass kernel for GaussianProcessEmbeddingHead.

The reference computes:
    mu     = x @ W_mu.T + b_mu                      (B,N,E)
    sigma  = exp(0.5*(x @ W_logvar.T + b_logvar))   (B,N,E)
    K      = RBF kernel matrix (B,N,N)  -- only its DIAGONAL is used,
             and dist_ii == 0 exactly, so cov_diag == 1 and the (B,N,N)
             work is mathematically dead. sigma_adjusted == sigma.
    return (mu, sigma_adjusted)

Strategy: data-parallel over batch B=8, one batch element per NeuronCore.
Per core: two linear heads over x_b [2048,1024] in bf16 (absmax-scaled
error vs the f32 reference: mu 3.9e-3, sigma 3.5e-3 -- well under the
2e-2 gate). The PE streams one output column per cycle, so the floor is
   2 heads * (2048*512 outputs / 128 lanes) * (1024/128 k-tiles)
   = 131072 cycles ~= 54.6 us @ 2.4 GHz.
(fp8 DoubleRow halves column passes per chain but needs >= 2 chains for
accuracy -- measured on HW: no faster than bf16, 10x worse error.)

Everything else is arranged to hide behind that stream:
 - x / W are transposed, bf16-cast AND partition-packed on host, so
   every DMA moves per-partition-contiguous slabs. No on-device
   transposes or casts.
 - Outputs are produced transposed ([E, N], partition = embedding), so
   each PSUM tile needs exactly ONE epilogue op with the bias fed
   through the per-partition port:
     sigma = Exp(PSUM * 0.5 + 0.5*b_lv[e])  on the Scalar engine
     mu    = PSUM + b_mu[e]                 on the Vector engine
   both writing bf16; host un-transposes and upcasts.
 - Loop nest: head -> token-chunk -> e-block -> k-tile. Each DMA ring
   sustains only ~150 GB/s, so the load schedule is ring-aware and
   paced to the consumption order: W is packed per e-block (one 256 KB
   slab per 1.7 us of compute, streamed on the scalar ring just ahead
   of the eb-inner loop), the first x chunk is split across the sync
   and gpsimd rings so the PE starts after ~10.5 us, and later chunks
   arrive with multi-us margin. lv stores ride sync, mu stores scalar.
 - Dummy warmup matmuls hold the PE's clock p-state up through the DMA
   lead-in; the final group is tapered (256/128/128 cols) so the
   serialized end-of-kernel epilogue is short.
"""
import os
import sys

import numpy as np

try:
    import concourse.bass as bass  # noqa: F401
except Exception:  # pragma: no cover - path fallback for fresh dirs
    for p in ("/opt/trn_rl_repo", os.path.expanduser("~/.axon_site/_ro/trn_rl_repo")):
        if os.path.isdir(p) and p not in sys.path:
            sys.path.insert(0, p)
    import concourse.bass as bass

import ml_dtypes
import concourse.mybir as mybir
from concourse import bacc
from concourse.bass_utils import run_bass_kernel_spmd
from concourse.tile import TileContext

B, N, D, E = 8, 2048, 1024, 512
P = 128
KT = D // P          # 8 k-tiles
EB = E // P          # 4 embedding blocks
TC = N // 512        # 4 token chunks of 512
F32, BF16 = mybir.dt.float32, mybir.dt.bfloat16

_NC = None


def _build():
    nc = bacc.Bacc()
    # x packed on host as [p][c][kt][t] -> [P, KT*N]
    xP = nc.declare_dram_parameter("xP", [P, KT * N], BF16, isOutput=False)
    # weights packed as [p][eb][kt][e'] -> [P, EB*KT*128] (eb slabs contiguous)
    wlv = nc.declare_dram_parameter("wlv", [P, E * KT], BF16, isOutput=False)
    wmu = nc.declare_dram_parameter("wmu", [P, E * KT], BF16, isOutput=False)
    # biases arranged [P, EB]: element (p, eb) = bias[eb*128 + p]
    bmu = nc.declare_dram_parameter("bmu", [P, EB], F32, isOutput=False)
    blv = nc.declare_dram_parameter("blv", [P, EB], F32, isOutput=False)  # 0.5*b
    muT = nc.declare_dram_parameter("muT", [E, N], BF16, isOutput=True)
    sgT = nc.declare_dram_parameter("sgT", [E, N], BF16, isOutput=True)

    with TileContext(nc) as tc:
        with (
            tc.tile_pool(name="const", bufs=1) as cpool,
            tc.tile_pool(name="out", bufs=6) as opool,
            tc.tile_pool(name="ps", bufs=4, space="PSUM") as psum,
        ):
            x_sb = [
                cpool.tile([P, KT, 512], BF16, name=f"x_sb{c}") for c in range(TC)
            ]
            wlv_sb = cpool.tile([P, KT, E], BF16)
            wmu_sb = cpool.tile([P, KT, E], BF16)
            blv_sb = cpool.tile([P, EB], F32)
            bmu_sb = cpool.tile([P, EB], F32)
            warm = cpool.tile([P, P], BF16)

            wlv_r = wlv[:, :].rearrange("p (kt e) -> p kt e", kt=KT)
            wmu_r = wmu[:, :].rearrange("p (kt e) -> p kt e", kt=KT)

            def xslab(c):
                off = c * 512 * KT
                return xP[:, off : off + 512 * KT].rearrange(
                    "p (kt t) -> p kt t", kt=KT
                )

            # Warmup: hold the PE busy through the DMA lead-in so the clock
            # p-state is ramped when the real stream starts.
            nc.vector.memset(warm, 0)
            wps = psum.tile([P, P], F32, tag="warm", bufs=1)
            for i in range(16):
                nc.tensor.matmul(
                    wps, warm[:, :], warm[:, :], start=(i == 0), stop=(i == 15)
                )

            # Load schedule: x chunk-0 halves land first on the sync and
            # gpsimd rings (sync starts transfers fastest), wlv k-tiles
            # stream on scalar (0-3) and sync (4-7) just ahead of the
            # eb0 pass, later x chunks follow on gpsimd one per 6.9 us.
            nc.sync.dma_start(out=x_sb[0][:, 0:4, :], in_=xslab(0)[:, 0:4, :])
            nc.gpsimd.dma_start(out=x_sb[0][:, 4:KT, :], in_=xslab(0)[:, 4:KT, :])
            nc.scalar.dma_start(out=wlv_sb[:, 0:2, :], in_=wlv_r[:, 0:2, :])
            nc.scalar.dma_start(out=wlv_sb[:, 2:4, :], in_=wlv_r[:, 2:4, :])
            nc.sync.dma_start(out=wlv_sb[:, 4:KT, :], in_=wlv_r[:, 4:KT, :])
            nc.sync.dma_start(out=blv_sb, in_=blv[:, :])
            nc.sync.dma_start(out=bmu_sb, in_=bmu[:, :])
            for c in range(1, TC):
                nc.gpsimd.dma_start(out=x_sb[c], in_=xslab(c))
            nc.sync.dma_start(out=wmu_sb, in_=wmu_r[:, :, :])

            EXP = mybir.ActivationFunctionType.Exp

            def group(hname, w_sb, outdram, bias_sb, c, eb, o0, ow):
                """One PSUM group: out columns [o0:o0+ow) of (head, chunk, eb)."""
                cs = slice(c * 512 + o0, c * 512 + o0 + ow)
                es = slice(eb * P, (eb + 1) * P)
                ps = psum.tile([P, ow], F32, tag="ps", name=f"ps_{hname}{c}{eb}_{o0}")
                for kt in range(KT):
                    nc.tensor.matmul(
                        ps,
                        w_sb[:, kt, es],
                        x_sb[c][:, kt, o0 : o0 + ow],
                        start=(kt == 0),
                        stop=(kt == KT - 1),
                    )
                o = opool.tile([P, ow], BF16, tag="o", name=f"o_{hname}{c}{eb}_{o0}")
                if hname == "lv":
                    nc.scalar.activation(
                        o, ps, EXP, bias=bias_sb[:, eb : eb + 1], scale=0.5
                    )
                    nc.sync.dma_start(out=outdram[es, cs], in_=o)
                else:
                    nc.vector.tensor_scalar_add(o, ps, bias_sb[:, eb : eb + 1])
                    nc.scalar.dma_start(out=outdram[es, cs], in_=o)

            heads = [("lv", wlv_sb, sgT, blv_sb), ("mu", wmu_sb, muT, bmu_sb)]
            for hname, w_sb, outdram, bias_sb in heads:
                last_head = hname == "mu"
                for c in range(TC):
                    for eb in range(EB):
                        if last_head and c == TC - 1 and eb == EB - 1:
                            # Taper the final group: short serialized tail.
                            for o0, ow in [(0, 256), (256, 128), (384, 128)]:
                                group(hname, w_sb, outdram, bias_sb, c, eb, o0, ow)
                        else:
                            group(hname, w_sb, outdram, bias_sb, c, eb, 0, 512)
    nc.compile()
    return nc


def _pack_x(xb):
    """xb [N, D] f32 -> [P, KT*N] bf16 packed as [p][c][kt][t]."""
    xt = xb.T.astype(ml_dtypes.bfloat16).reshape(KT, P, TC, 512)  # [kt, p, c, t]
    return np.ascontiguousarray(xt.transpose(1, 2, 0, 3).reshape(P, KT * N))


def _pack_w(W):
    """W [E, D] f32 -> [P, KT*E] bf16 packed as [p][kt][e]."""
    wt = W.astype(np.float32).T.astype(ml_dtypes.bfloat16)   # [D, E]
    v = wt.reshape(KT, P, E)
    return np.ascontiguousarray(v.transpose(1, 0, 2).reshape(P, KT * E))


def run(x, W_mu, b_mu, W_logvar, b_logvar, trace=False, **trace_kwargs):
    global _NC
    if _NC is None:
        _NC = _build()

    x = np.asarray(x, dtype=np.float32)
    wlv_h = _pack_w(np.asarray(W_logvar))
    wmu_h = _pack_w(np.asarray(W_mu))
    bmu_h = np.ascontiguousarray(np.asarray(b_mu, dtype=np.float32).reshape(EB, P).T)
    blv_h = np.ascontiguousarray(
        (0.5 * np.asarray(b_logvar, dtype=np.float32)).reshape(EB, P).T
    )

    in_maps = [
        {
            "xP": _pack_x(x[b]),
            "wlv": wlv_h,
            "wmu": wmu_h,
            "bmu": bmu_h,
            "blv": blv_h,
        }
        for b in range(B)
    ]
    res = run_bass_kernel_spmd(
        _NC, in_maps, core_ids=list(range(B)), trace=trace, **trace_kwargs
    )
    mu = np.stack(
        [res.results[b]["muT"].reshape(E, N).T.astype(np.float32) for b in range(B)]
    )
    sigma = np.stack(
        [res.results[b]["sgT"].reshape(E, N).T.astype(np.float32) for b in range(B)]
    )
    return (mu, sigma), res


def kernel(x, W_mu, b_mu, W_logvar, b_logvar):
    (mu, sigma), _ = run(x, W_mu, b_mu, W_logvar, b_logvar, trace=False)
    return mu, sigma


# revision 15
# speedup vs baseline: 1.0765x; 1.0765x over previous
"""Trainium2 Bass kernel for GaussianProcessEmbeddingHead.

The reference computes:
    mu     = x @ W_mu.T + b_mu                      (B,N,E)
    sigma  = exp(0.5*(x @ W_logvar.T + b_logvar))   (B,N,E)
    K      = RBF kernel matrix (B,N,N)  -- only its DIAGONAL is used,
             and dist_ii == 0 exactly, so cov_diag == 1 and the (B,N,N)
             work is mathematically dead. sigma_adjusted == sigma.
    return (mu, sigma_adjusted)

Strategy: data-parallel over batch B=8, one batch element per NeuronCore.
Per core: two linear heads over x_b [2048,1024] in bf16 (absmax-scaled
error vs the f32 reference: mu 3.9e-3, sigma 3.5e-3 -- well under the
2e-2 gate). The PE streams one output column per cycle, so the floor is
   2 heads * (2048*512 outputs / 128 lanes) * (1024/128 k-tiles)
   = 131072 cycles ~= 54.6 us @ 2.4 GHz.
(fp8 DoubleRow halves column passes per chain but needs >= 2 chains for
accuracy -- measured on HW: no faster than bf16, 10x worse error.)

Everything else is arranged to hide behind that stream:
 - x / W are transposed, bf16-cast AND partition-packed on host, so
   every DMA moves per-partition-contiguous slabs. No on-device
   transposes or casts.
 - Outputs are produced transposed ([E, N], partition = embedding), so
   each PSUM tile needs exactly ONE epilogue op with the bias fed
   through the per-partition port:
     sigma = Exp(PSUM * 0.5 + 0.5*b_lv[e])  on the Scalar engine
     mu    = PSUM + b_mu[e]                 on the Vector engine
   both writing bf16; host un-transposes and upcasts.
 - Loop nest: head -> token-chunk -> e-block -> k-tile. Each DMA ring
   sustains only ~150 GB/s, so the load schedule is ring-aware and
   paced to the consumption order: W is packed per e-block (one 256 KB
   slab per 1.7 us of compute, streamed on the scalar ring just ahead
   of the eb-inner loop), the first x chunk is split across the sync
   and gpsimd rings so the PE starts after ~10.5 us, and later chunks
   arrive with multi-us margin. lv stores ride sync, mu stores scalar.
 - Dummy warmup matmuls hold the PE's clock p-state up through the DMA
   lead-in; the final group is tapered (256/128/128 cols) so the
   serialized end-of-kernel epilogue is short.
"""
import os
import sys

import numpy as np

try:
    import concourse.bass as bass  # noqa: F401
except Exception:  # pragma: no cover - path fallback for fresh dirs
    for p in ("/opt/trn_rl_repo", os.path.expanduser("~/.axon_site/_ro/trn_rl_repo")):
        if os.path.isdir(p) and p not in sys.path:
            sys.path.insert(0, p)
    import concourse.bass as bass

import ml_dtypes
import concourse.mybir as mybir
from concourse import bacc
from concourse.bass_utils import run_bass_kernel_spmd
from concourse.tile import TileContext

B, N, D, E = 8, 2048, 1024, 512
P = 128
KT = D // P          # 8 k-tiles
EB = E // P          # 4 embedding blocks
TC = N // 512        # 4 token chunks of 512
F32, BF16 = mybir.dt.float32, mybir.dt.bfloat16

_NC = None


def _build():
    nc = bacc.Bacc()
    # x packed on host as [p][c][kt][t] -> [P, KT*N]
    xP = nc.declare_dram_parameter("xP", [P, KT * N], BF16, isOutput=False)
    # weights packed as [p][eb][kt][e'] -> [P, EB*KT*128] (eb slabs contiguous)
    wlv = nc.declare_dram_parameter("wlv", [P, E * KT], BF16, isOutput=False)
    wmu = nc.declare_dram_parameter("wmu", [P, E * KT], BF16, isOutput=False)
    # biases arranged [P, EB]: element (p, eb) = bias[eb*128 + p]
    bmu = nc.declare_dram_parameter("bmu", [P, EB], F32, isOutput=False)
    blv = nc.declare_dram_parameter("blv", [P, EB], F32, isOutput=False)  # 0.5*b
    muT = nc.declare_dram_parameter("muT", [E, N], BF16, isOutput=True)
    sgT = nc.declare_dram_parameter("sgT", [E, N], BF16, isOutput=True)

    with TileContext(nc) as tc:
        with (
            tc.tile_pool(name="const", bufs=1) as cpool,
            tc.tile_pool(name="out", bufs=6) as opool,
            tc.tile_pool(name="ps", bufs=4, space="PSUM") as psum,
        ):
            x_sb = [
                cpool.tile([P, KT, 512], BF16, name=f"x_sb{c}") for c in range(TC)
            ]
            wlv_sb = cpool.tile([P, KT, E], BF16)
            wmu_sb = cpool.tile([P, KT, E], BF16)
            blv_sb = cpool.tile([P, EB], F32)
            bmu_sb = cpool.tile([P, EB], F32)
            warm = cpool.tile([P, P], BF16)

            wlv_r = wlv[:, :].rearrange("p (kt e) -> p kt e", kt=KT)
            wmu_r = wmu[:, :].rearrange("p (kt e) -> p kt e", kt=KT)

            def xslab(c):
                off = c * 512 * KT
                return xP[:, off : off + 512 * KT].rearrange(
                    "p (kt t) -> p kt t", kt=KT
                )

            # Warmup: hold the PE busy through the DMA lead-in so the clock
            # p-state is ramped when the real stream starts.
            nc.vector.memset(warm, 0)
            wps = psum.tile([P, P], F32, tag="warm", bufs=1)
            for i in range(16):
                nc.tensor.matmul(
                    wps, warm[:, :], warm[:, :], start=(i == 0), stop=(i == 15)
                )

            # Load schedule (empirically the best of many tried; "cleverer"
            # ring-balanced variants all produced mid-stream stalls): first
            # wlv k-tiles 0-1 on the scalar ring, x chunk 0 in two slabs on
            # gpsimd, rest of wlv + biases on sync, then the remaining x
            # chunks stream on gpsimd one per 6.9 us of compute, wmu on sync.
            nc.scalar.dma_start(out=wlv_sb[:, 0:2, :], in_=wlv_r[:, 0:2, :])
            nc.gpsimd.dma_start(out=x_sb[0][:, 0:4, :], in_=xslab(0)[:, 0:4, :])
            nc.gpsimd.dma_start(out=x_sb[0][:, 4:KT, :], in_=xslab(0)[:, 4:KT, :])
            nc.sync.dma_start(out=wlv_sb[:, 2:KT, :], in_=wlv_r[:, 2:KT, :])
            nc.sync.dma_start(out=blv_sb, in_=blv[:, :])
            nc.sync.dma_start(out=bmu_sb, in_=bmu[:, :])
            for c in range(1, TC):
                nc.gpsimd.dma_start(out=x_sb[c], in_=xslab(c))
            nc.sync.dma_start(out=wmu_sb, in_=wmu_r[:, :, :])

            EXP = mybir.ActivationFunctionType.Exp

            def group(hname, w_sb, outdram, bias_sb, c, eb, o0, ow):
                """One PSUM group: out columns [o0:o0+ow) of (head, chunk, eb)."""
                cs = slice(c * 512 + o0, c * 512 + o0 + ow)
                es = slice(eb * P, (eb + 1) * P)
                ps = psum.tile([P, ow], F32, tag="ps", name=f"ps_{hname}{c}{eb}_{o0}")
                for kt in range(KT):
                    nc.tensor.matmul(
                        ps,
                        w_sb[:, kt, es],
                        x_sb[c][:, kt, o0 : o0 + ow],
                        start=(kt == 0),
                        stop=(kt == KT - 1),
                    )
                o = opool.tile([P, ow], BF16, tag="o", name=f"o_{hname}{c}{eb}_{o0}")
                if hname == "lv":
                    nc.scalar.activation(
                        o, ps, EXP, bias=bias_sb[:, eb : eb + 1], scale=0.5
                    )
                    nc.sync.dma_start(out=outdram[es, cs], in_=o)
                else:
                    nc.vector.tensor_scalar_add(o, ps, bias_sb[:, eb : eb + 1])
                    nc.scalar.dma_start(out=outdram[es, cs], in_=o)

            heads = [("lv", wlv_sb, sgT, blv_sb), ("mu", wmu_sb, muT, bmu_sb)]
            for hname, w_sb, outdram, bias_sb in heads:
                last_head = hname == "mu"
                for c in range(TC):
                    for eb in range(EB):
                        if last_head and c == TC - 1 and eb == EB - 1:
                            # Taper the final group: short serialized tail.
                            for o0, ow in [(0, 256), (256, 128), (384, 128)]:
                                group(hname, w_sb, outdram, bias_sb, c, eb, o0, ow)
                        else:
                            group(hname, w_sb, outdram, bias_sb, c, eb, 0, 512)
    nc.compile()
    return nc


def _pack_x(xb):
    """xb [N, D] f32 -> [P, KT*N] bf16 packed as [p][c][kt][t]."""
    xt = xb.T.astype(ml_dtypes.bfloat16).reshape(KT, P, TC, 512)  # [kt, p, c, t]
    return np.ascontiguousarray(xt.transpose(1, 2, 0, 3).reshape(P, KT * N))


def _pack_w(W):
    """W [E, D] f32 -> [P, KT*E] bf16 packed as [p][kt][e]."""
    wt = W.astype(np.float32).T.astype(ml_dtypes.bfloat16)   # [D, E]
    v = wt.reshape(KT, P, E)
    return np.ascontiguousarray(v.transpose(1, 0, 2).reshape(P, KT * E))


def run(x, W_mu, b_mu, W_logvar, b_logvar, trace=False, **trace_kwargs):
    global _NC
    if _NC is None:
        _NC = _build()

    x = np.asarray(x, dtype=np.float32)
    wlv_h = _pack_w(np.asarray(W_logvar))
    wmu_h = _pack_w(np.asarray(W_mu))
    bmu_h = np.ascontiguousarray(np.asarray(b_mu, dtype=np.float32).reshape(EB, P).T)
    blv_h = np.ascontiguousarray(
        (0.5 * np.asarray(b_logvar, dtype=np.float32)).reshape(EB, P).T
    )

    in_maps = [
        {
            "xP": _pack_x(x[b]),
            "wlv": wlv_h,
            "wmu": wmu_h,
            "bmu": bmu_h,
            "blv": blv_h,
        }
        for b in range(B)
    ]
    res = run_bass_kernel_spmd(
        _NC, in_maps, core_ids=list(range(B)), trace=trace, **trace_kwargs
    )
    mu = np.stack(
        [res.results[b]["muT"].reshape(E, N).T.astype(np.float32) for b in range(B)]
    )
    sigma = np.stack(
        [res.results[b]["sgT"].reshape(E, N).T.astype(np.float32) for b in range(B)]
    )
    return (mu, sigma), res


def kernel(x, W_mu, b_mu, W_logvar, b_logvar):
    (mu, sigma), _ = run(x, W_mu, b_mu, W_logvar, b_logvar, trace=False)
    return mu, sigma
